# revision 20
# baseline (speedup 1.0000x reference)
"""ConcatCritic MLP over the B^2 pair grid, Trainium2 Bass/Tile kernel.

out[i, j] = softplus(f(x[i], y[j])) where f is a 4-layer MLP on
concat(x, y). Reference pair grid: pairs[a*n+b] = concat(x[b], y[a]),
scores.reshape(n,n).T -> out.

Key factorization: layer 1 is linear in the concat, so
  h1[a,b] = relu(x[b] @ W1top + y[a] @ W1bot + b1)
with W1top = W1[:128], W1bot = W1[128:]. The [B^2, 256] layer-1 matmul
collapses into two tiny matmuls plus a per-partition broadcast add.

Layout: activations kept transposed [features, batch] so every layer's
matmul (weights stationary as lhsT [K, M]) writes the next layer's rhs
directly: out[m=feat, n=j] = sum_k W[k, m] * hT[k, j].

Perf structure (vs the fp32r baseline):
- fp16 matmul operands (same PE stream rate, but FWL weight loads and
  2x DVE read modes; rel err ~1e-3, gate is 2e-2).
- L4 (score row) as two col-tiled M=1 matmuls at tile_position (0,0)
  and (0,32) running concurrently in different column groups -> ~1 PE
  slot instead of 2.
- Per-step drain is a single DMA of the two psum score rows into a
  staging tile; softplus runs batched per 32-row half (one ACT op) --
  no per-step EXP, and Identity/Relu/Softplus all live in one ACT
  table (softplus_and_others) so there are zero mid-kernel table
  swaps.
- PE warm-up: ~40 junk matmuls issued during the input-DMA window keep
  the HAM activity monitor busy so the real stream starts at 2.4 GHz
  (baseline ran its first ~11 us of matmuls at 1.2 GHz).
- Inputs packed into 4 DRAM blobs, triggered on 4 different engine
  queues in parallel.

Sharding: core c owns y rows [c*64, (c+1)*64); computes block
V_c[il, j] = f(x[j], y[c*64+il]) of shape [64, 512]. Host gathers
V = concat(V_c) and returns V.T.

Self-contained: hardcodes shapes; imports concourse from the system repo.
"""

import os
import sys

import numpy as np


def _import_concourse():
    try:
        import concourse  # noqa: F401
        return
    except ImportError:
        pass
    for p in ("/opt/trn_rl_repo", "/root/.axon_site/_ro/trn_rl_repo"):
        if os.path.isdir(p) and p not in sys.path:
            sys.path.insert(0, p)
    import concourse  # noqa: F401


_import_concourse()

import concourse.bacc as bacc  # noqa: E402
import concourse.tile as tile  # noqa: E402
from concourse import mybir  # noqa: E402
from concourse.bass_utils import run_bass_kernel_spmd  # noqa: E402

B = 512          # batch (pair-grid side)
D = 128          # input dim per tensor
H = 256          # hidden dim
NCORES = 8
RB = B // NCORES  # 64 y-rows per core
F32 = mybir.dt.float32
MM_DT = mybir.dt.float16
NP_DT = np.float16
N_WARM = 5       # junk N=512 matmuls to warm the PE HAM before the real stream


def _emit(tc, nc, d, out_d):
    AF = mybir.ActivationFunctionType
    OP = mybir.AluOpType
    from contextlib import ExitStack

    with ExitStack() as ctx:
        const = ctx.enter_context(tc.tile_pool(name="const", bufs=1))
        hpool = ctx.enter_context(tc.tile_pool(name="h", bufs=2))
        psum = ctx.enter_context(tc.tile_pool(name="psum", bufs=1, space="PSUM"))

        # ---- PE warm-up: matmuls on a zeroed junk tile, issued while the
        # input DMAs are in flight. Keeps the HAM activity window busy so
        # the real matmul stream runs at 2.4 GHz from the start.
        junk = const.tile([128, B], MM_DT, tag="junk", name="junk")
        nc.gpsimd.memset(junk[:], 0.0)
        for k in range(N_WARM):
            pw = psum.tile([1, B], F32, tag="l4", name=f"warm{k}", bufs=2)
            nc.tensor.matmul(pw[:], lhsT=junk[:, 0:1], rhs=junk[:],
                             start=True, stop=True)

        # ---- packed input loads, one blob per engine DMA queue.
        A = const.tile([128, 320], MM_DT, tag="blobA", name="A")
        nc.sync.dma_start(out=A[:], in_=d["blobA"][:])
        Bt = const.tile([128, 768], MM_DT, tag="blobB", name="Bt")
        nc.scalar.dma_start(out=Bt[:], in_=d["blobB"][:])
        Dt = const.tile([128, 7], F32, tag="blobD", name="Dt")
        nc.gpsimd.dma_start(out=Dt[:], in_=d["blobD"][:])
        C = const.tile([128, 1026], MM_DT, tag="blobC", name="C")
        nc.gpsimd.dma_start(out=C[:], in_=d["blobC"][:])

        yT = A[:, 0:64]
        w1b = A[:, 64:320]
        w1t = Bt[:, 0:256]
        xT = Bt[:, 256:768]

        def w2(kc, mc):
            return C[:, kc * 256 + mc * 128: kc * 256 + (mc + 1) * 128]

        def w3(kc, mc):
            return C[:, 512 + kc * 256 + mc * 128: 512 + kc * 256 + (mc + 1) * 128]

        w4c = [C[:, 1024:1025], C[:, 1025:1026]]
        b1c = [Dt[:, 0:1], Dt[:, 1:2]]
        b2c = [Dt[:, 2:3], Dt[:, 3:4]]
        b3c = [Dt[:, 4:5], Dt[:, 5:6]]
        b4 = Dt[:, 6:7]

        # ---- preamble: xa[oc] = (x @ W1top)^T + b1 (bias folded here),
        #                yb[oc] = (y_slice @ W1bot)^T
        # Preamble psum tiles borrow the main-loop l2/l3 tags (no extra banks).
        yb = []
        for oc in range(2):
            ms = slice(oc * 128, (oc + 1) * 128)
            pyb = psum.tile([128, B], F32, tag="l3", name=f"pyb{oc}", bufs=3)
            nc.tensor.matmul(pyb[:, 0:RB], lhsT=w1b[:, ms], rhs=yT, start=True, stop=True)
            ybt = const.tile([128, RB], F32, tag=f"yb{oc}", name=f"yb{oc}")
            nc.vector.tensor_copy(ybt[:], pyb[:, 0:RB])
            yb.append(ybt)
        xa = []
        for oc in range(2):
            ms = slice(oc * 128, (oc + 1) * 128)
            pxa = psum.tile([128, B], F32, tag="l2", name=f"pxa{oc}", bufs=3)
            nc.tensor.matmul(pxa[:], lhsT=w1t[:, ms], rhs=xT, start=True, stop=True)
            xat = const.tile([128, B], MM_DT, tag=f"xa{oc}", name=f"xa{oc}")
            nc.scalar.activation(xat[:], pxa[:], AF.Identity, bias=b1c[oc])
            xa.append(xat)

        staging = const.tile([RB, B], MM_DT, tag="stage", name="stage")
        fin = const.tile([RB, B], F32, tag="fin", name="fin")

        # Software-pipelined emission, deep lags so every PE instruction's
        # inputs are produced >= 1 full step earlier (sem latency ~0.4us
        # never exposes): per step t the PE stream is
        #   L4MM(t-4) | L3MM(t-2) | L2MM(t)
        # with L1(t+2) leading 2 steps ahead on DVE.
        # Engine split per step: DVE: 2x L1 + 2x L3epi; ACT: 2x L2epi + exp.
        # Score rows drain by DMA; ln(1+e) batched at the end.
        # PSUM banks: tag "l2" bufs=3 + tag "l3" bufs=3 (each layer's two
        # chunks share a 3-deep rotation -> banks reused 1.5 steps later,
        # long after the epilogue freed them) + l4 bufs=2 = 8 exactly.
        h1s, h2s, h3s, p4s = {}, {}, {}, {}

        def emit_l1(i):
            for oc in range(2):
                t = hpool.tile([128, B], MM_DT, tag=f"h1_{oc}", name=f"h1_{oc}_{i}", bufs=4)
                nc.vector.tensor_scalar(
                    t[:], xa[oc][:], yb[oc][:, i:i + 1], 0.0, OP.add, OP.max
                )
                h1s[i, oc] = t

        def emit_l2mm(i):
            for mc in range(2):
                p = psum.tile([128, B], F32, tag="l2", name=f"p2_{mc}_{i}", bufs=3)
                for kc in range(2):
                    nc.tensor.matmul(
                        p[:], lhsT=w2(kc, mc), rhs=h1s[i, kc][:],
                        start=(kc == 0), stop=(kc == 1),
                    )
                h2s[i, mc] = p  # psum handle; epi converts to SBUF below

        def emit_l2epi(i):
            for mc in range(2):
                p = h2s[i, mc]
                t = hpool.tile([128, B], MM_DT, tag=f"h2_{mc}", name=f"h2_{mc}_{i}", bufs=3)
                nc.scalar.activation(t[:], p[:], AF.Relu, bias=b2c[mc])
                h2s[i, mc] = t

        def emit_l3mm(i):
            for mc in range(2):
                p = psum.tile([128, B], F32, tag="l3", name=f"p3_{mc}_{i}", bufs=3)
                for kc in range(2):
                    nc.tensor.matmul(
                        p[:], lhsT=w3(kc, mc), rhs=h2s[i, kc][:],
                        start=(kc == 0), stop=(kc == 1),
                    )
                h3s[i, mc] = p

        def emit_l3epi(i):
            # Both on DVE: ACT is the busier engine (2x L2epi + exp drain).
            for mc in range(2):
                p = h3s[i, mc]
                t = hpool.tile([128, B], MM_DT, tag=f"h3_{mc}", name=f"h3_{mc}_{i}", bufs=3)
                nc.vector.tensor_scalar(t[:], p[:], b3c[mc], 0.0, OP.add, OP.max)
                h3s[i, mc] = t
            del h2s[i, 0], h2s[i, 1]

        def emit_l4mm(i):
            p4 = psum.tile([1, B], F32, tag="l4", name=f"p4_{i}", bufs=2)
            for kc in range(2):
                nc.tensor.matmul(
                    p4[:], lhsT=w4c[kc], rhs=h3s[i, kc][:],
                    start=(kc == 0), stop=(kc == 1),
                )
            p4s[i] = p4
            del h3s[i, 0], h3s[i, 1]

        def emit_drain(i):
            # Evacuate the psum score row as fp16 e^(s+b4) (the psum->SBUF
            # hop must be an engine op anyway, so exp rides along free), then
            # DMA it into staging row i (DMA targets any partition; engines
            # only write partition bases {0,32,64,96}).
            s4 = hpool.tile([1, B], MM_DT, tag="s4", name=f"s4_{i}", bufs=3)
            nc.scalar.activation(s4[:], p4s.pop(i)[:], AF.Exp, bias=b4[0:1])
            nc.sync.dma_start(out=staging[i:i + 1, :], in_=s4[:])

        # ---- tail: one batched ln(1+e) over all 64 score rows. Identity/
        # Relu/Exp share the initially-loaded table; only this final Ln
        # forces the one table swap of the kernel (it overlaps the last
        # drain DMA).
        def emit_tail():
            nc.scalar.activation(fin[:], staging[:], AF.Ln, bias=1.0)
            nc.sync.dma_start(out=out_d[:], in_=fin[:])

        emit_l1(0)
        emit_l1(1)
        for t in range(RB + 3):
            if t >= 3:
                emit_l4mm(t - 3)
                emit_drain(t - 3)
            if 2 <= t < RB + 2:
                emit_l3mm(t - 2)
                emit_l3epi(t - 2)
            if t + 2 < RB:
                emit_l1(t + 2)
            if t < RB:
                emit_l2mm(t)
                emit_l2epi(t)
        emit_tail()


def _build_program():
    nc = bacc.Bacc("TRN2", target_bir_lowering=False, debug=False, enable_asserts=False)
    d = {}
    for name, shape, dt in [
        ("blobA", (128, 320), MM_DT),
        ("blobB", (128, 768), MM_DT),
        ("blobC", (128, 1026), MM_DT),
        ("blobD", (128, 7), F32),
    ]:
        d[name] = nc.dram_tensor(name, list(shape), dt, kind="ExternalInput").ap()
    out_d = nc.dram_tensor("out", [RB, B], F32, kind="ExternalOutput").ap()
    with tile.TileContext(nc) as tc:
        _emit(tc, nc, d, out_d)
    nc.compile()
    return nc


_PROGRAM = None


def _get_program():
    global _PROGRAM
    if _PROGRAM is None:
        _PROGRAM = _build_program()
    return _PROGRAM


def _make_in_maps(x, y, W1, b1, W2, b2, W3, b3, W4, b4):
    f = np.float32
    x = np.asarray(x, f)
    y = np.asarray(y, f)
    W1 = np.asarray(W1, f)
    W2 = np.asarray(W2, f)
    W3 = np.asarray(W3, f)
    W4 = np.asarray(W4, f).reshape(H, 1)
    b1 = np.asarray(b1, f).reshape(H)
    b2 = np.asarray(b2, f).reshape(H)
    b3 = np.asarray(b3, f).reshape(H)
    b4v = np.asarray(b4, f).reshape(-1)[0]

    blobB = np.ascontiguousarray(
        np.concatenate([W1[:D], x.T], axis=1), dtype=NP_DT)
    blobC = np.ascontiguousarray(
        np.concatenate([W2[:128], W2[128:], W3[:128], W3[128:],
                        W4[:128], W4[128:]], axis=1), dtype=NP_DT)
    blobD = np.ascontiguousarray(
        np.stack([b1[:128], b1[128:], b2[:128], b2[128:],
                  b3[:128], b3[128:], np.full(128, b4v, f)], axis=1), dtype=f)
    W1b = W1[D:]
    in_maps = []
    for c in range(NCORES):
        blobA = np.ascontiguousarray(
            np.concatenate([y[c * RB:(c + 1) * RB].T, W1b], axis=1), dtype=NP_DT)
        in_maps.append({"blobA": blobA, "blobB": blobB, "blobC": blobC,
                        "blobD": blobD})
    return in_maps


def _run(inputs, trace=False, trace_cores=None):
    nc = _get_program()
    in_maps = _make_in_maps(**inputs)
    res = run_bass_kernel_spmd(
        nc, in_maps, list(range(NCORES)), trace=trace, trace_cores=trace_cores,
    )
    V = np.concatenate([res.results[c]["out"] for c in range(NCORES)], axis=0)
    out = np.ascontiguousarray(V.T, dtype=np.float32)
    return out, res


def kernel(**inputs):
    out, _ = _run(inputs, trace=False)
    return out


# revision 21
# speedup vs baseline: 1.0721x; 1.0721x over previous
"""ConcatCritic MLP over the B^2 pair grid, Trainium2 Bass/Tile kernel.

out[i, j] = softplus(f(x[i], y[j])) where f is a 4-layer MLP on
concat(x, y). Reference pair grid: pairs[a*n+b] = concat(x[b], y[a]),
scores.reshape(n,n).T -> out.

Key factorization: layer 1 is linear in the concat, so
  h1[a,b] = relu(x[b] @ W1top + y[a] @ W1bot + b1)
with W1top = W1[:128], W1bot = W1[128:]. The [B^2, 256] layer-1 matmul
collapses into two tiny matmuls plus a per-partition broadcast add.

Layout: activations kept transposed [features, batch] so every layer's
matmul (weights stationary as lhsT [K, M]) writes the next layer's rhs
directly: out[m=feat, n=j] = sum_k W[k, m] * hT[k, j].

Perf structure (vs the fp32r baseline):
- fp16 matmul operands (same PE stream rate, but FWL weight loads and
  2x DVE read modes; rel err ~1e-3, gate is 2e-2).
- L4 (score row) as two col-tiled M=1 matmuls at tile_position (0,0)
  and (0,32) running concurrently in different column groups -> ~1 PE
  slot instead of 2.
- Per-step drain is a single DMA of the two psum score rows into a
  staging tile; softplus runs batched per 32-row half (one ACT op) --
  no per-step EXP, and Identity/Relu/Softplus all live in one ACT
  table (softplus_and_others) so there are zero mid-kernel table
  swaps.
- PE warm-up: ~40 junk matmuls issued during the input-DMA window keep
  the HAM activity monitor busy so the real stream starts at 2.4 GHz
  (baseline ran its first ~11 us of matmuls at 1.2 GHz).
- Inputs packed into 4 DRAM blobs, triggered on 4 different engine
  queues in parallel.

Sharding: core c owns y rows [c*64, (c+1)*64); computes block
V_c[il, j] = f(x[j], y[c*64+il]) of shape [64, 512]. Host gathers
V = concat(V_c) and returns V.T.

Self-contained: hardcodes shapes; imports concourse from the system repo.
"""

import os
import sys

import numpy as np


def _import_concourse():
    try:
        import concourse  # noqa: F401
        return
    except ImportError:
        pass
    for p in ("/opt/trn_rl_repo", "/root/.axon_site/_ro/trn_rl_repo"):
        if os.path.isdir(p) and p not in sys.path:
            sys.path.insert(0, p)
    import concourse  # noqa: F401


_import_concourse()

import concourse.bacc as bacc  # noqa: E402
import concourse.tile as tile  # noqa: E402
from concourse import mybir  # noqa: E402
from concourse.bass_utils import run_bass_kernel_spmd  # noqa: E402

B = 512          # batch (pair-grid side)
D = 128          # input dim per tensor
H = 256          # hidden dim
NCORES = 8
RB = B // NCORES  # 64 y-rows per core
F32 = mybir.dt.float32
MM_DT = mybir.dt.float16
NP_DT = np.float16
N_WARM = 5       # junk N=512 matmuls to warm the PE HAM before the real stream


def _emit(tc, nc, d, out_d):
    AF = mybir.ActivationFunctionType
    OP = mybir.AluOpType
    from contextlib import ExitStack

    with ExitStack() as ctx:
        const = ctx.enter_context(tc.tile_pool(name="const", bufs=1))
        hpool = ctx.enter_context(tc.tile_pool(name="h", bufs=2))
        psum = ctx.enter_context(tc.tile_pool(name="psum", bufs=1, space="PSUM"))

        # ---- PE warm-up: matmuls on a zeroed junk tile, issued while the
        # input DMAs are in flight. Keeps the HAM activity window busy so
        # the real matmul stream runs at 2.4 GHz from the start.
        junk = const.tile([128, B], MM_DT, tag="junk", name="junk")
        nc.gpsimd.memset(junk[:], 0.0)
        for k in range(N_WARM):
            pw = psum.tile([1, B], F32, tag="l4", name=f"warm{k}", bufs=2)
            nc.tensor.matmul(pw[:], lhsT=junk[:, 0:1], rhs=junk[:],
                             start=True, stop=True)

        # ---- packed input loads, one blob per engine DMA queue.
        A = const.tile([128, 320], MM_DT, tag="blobA", name="A")
        nc.sync.dma_start(out=A[:], in_=d["blobA"][:])
        Bt = const.tile([128, 768], MM_DT, tag="blobB", name="Bt")
        nc.scalar.dma_start(out=Bt[:], in_=d["blobB"][:])
        Dt = const.tile([128, 7], F32, tag="blobD", name="Dt")
        nc.gpsimd.dma_start(out=Dt[:], in_=d["blobD"][:])
        C = const.tile([128, 1026], MM_DT, tag="blobC", name="C")
        nc.gpsimd.dma_start(out=C[:], in_=d["blobC"][:])

        yT = A[:, 0:64]
        w1b = A[:, 64:320]
        w1t = Bt[:, 0:256]
        xT = Bt[:, 256:768]

        def w2(kc, mc):
            return C[:, kc * 256 + mc * 128: kc * 256 + (mc + 1) * 128]

        def w3(kc, mc):
            return C[:, 512 + kc * 256 + mc * 128: 512 + kc * 256 + (mc + 1) * 128]

        w4c = [C[:, 1024:1025], C[:, 1025:1026]]
        b1c = [Dt[:, 0:1], Dt[:, 1:2]]
        b2c = [Dt[:, 2:3], Dt[:, 3:4]]
        b3c = [Dt[:, 4:5], Dt[:, 5:6]]
        b4 = Dt[:, 6:7]

        # ---- preamble: xa[oc] = (x @ W1top)^T + b1 (bias folded here),
        #                yb[oc] = (y_slice @ W1bot)^T
        # Preamble psum tiles borrow the main-loop l2/l3 tags (no extra banks).
        yb = []
        for oc in range(2):
            ms = slice(oc * 128, (oc + 1) * 128)
            pyb = psum.tile([128, B], F32, tag="l3", name=f"pyb{oc}", bufs=3)
            nc.tensor.matmul(pyb[:, 0:RB], lhsT=w1b[:, ms], rhs=yT, start=True, stop=True)
            ybt = const.tile([128, RB], F32, tag=f"yb{oc}", name=f"yb{oc}")
            nc.vector.tensor_copy(ybt[:], pyb[:, 0:RB])
            yb.append(ybt)
        xa = []
        for oc in range(2):
            ms = slice(oc * 128, (oc + 1) * 128)
            pxa = psum.tile([128, B], F32, tag="l2", name=f"pxa{oc}", bufs=3)
            nc.tensor.matmul(pxa[:], lhsT=w1t[:, ms], rhs=xT, start=True, stop=True)
            xat = const.tile([128, B], MM_DT, tag=f"xa{oc}", name=f"xa{oc}")
            nc.scalar.activation(xat[:], pxa[:], AF.Identity, bias=b1c[oc])
            xa.append(xat)

        staging = const.tile([RB, B], MM_DT, tag="stage", name="stage")
        fin = const.tile([RB, B], F32, tag="fin", name="fin")

        # Software-pipelined emission, deep lags so every PE instruction's
        # inputs are produced >= 1 full step earlier (sem latency ~0.4us
        # never exposes): per step t the PE stream is
        #   L4MM(t-4) | L3MM(t-2) | L2MM(t)
        # with L1(t+2) leading 2 steps ahead on DVE.
        # Engine split per step: DVE: 2x L1 + 2x L3epi; ACT: 2x L2epi + exp.
        # Score rows drain by DMA; ln(1+e) batched at the end.
        # PSUM banks: tag "l2" bufs=3 + tag "l3" bufs=3 (each layer's two
        # chunks share a 3-deep rotation -> banks reused 1.5 steps later,
        # long after the epilogue freed them) + l4 bufs=2 = 8 exactly.
        h1s, h2s, h3s, p4s = {}, {}, {}, {}

        def emit_l1(i):
            for oc in range(2):
                t = hpool.tile([128, B], MM_DT, tag=f"h1_{oc}", name=f"h1_{oc}_{i}", bufs=4)
                nc.vector.tensor_scalar(
                    t[:], xa[oc][:], yb[oc][:, i:i + 1], 0.0, OP.add, OP.max
                )
                h1s[i, oc] = t

        def emit_l2mm(i):
            for mc in range(2):
                p = psum.tile([128, B], F32, tag="l2", name=f"p2_{mc}_{i}", bufs=3)
                for kc in range(2):
                    nc.tensor.matmul(
                        p[:], lhsT=w2(kc, mc), rhs=h1s[i, kc][:],
                        start=(kc == 0), stop=(kc == 1),
                    )
                h2s[i, mc] = p  # psum handle; epi converts to SBUF below

        def emit_l2epi(i):
            for mc in range(2):
                p = h2s[i, mc]
                t = hpool.tile([128, B], MM_DT, tag=f"h2_{mc}", name=f"h2_{mc}_{i}", bufs=3)
                nc.scalar.activation(t[:], p[:], AF.Relu, bias=b2c[mc])
                h2s[i, mc] = t

        def emit_l3mm(i):
            for mc in range(2):
                p = psum.tile([128, B], F32, tag="l3", name=f"p3_{mc}_{i}", bufs=3)
                for kc in range(2):
                    nc.tensor.matmul(
                        p[:], lhsT=w3(kc, mc), rhs=h2s[i, kc][:],
                        start=(kc == 0), stop=(kc == 1),
                    )
                h3s[i, mc] = p

        def emit_l3epi(i):
            # Both on DVE: ACT is the busier engine (2x L2epi + exp drain).
            for mc in range(2):
                p = h3s[i, mc]
                t = hpool.tile([128, B], MM_DT, tag=f"h3_{mc}", name=f"h3_{mc}_{i}", bufs=3)
                nc.vector.tensor_scalar(t[:], p[:], b3c[mc], 0.0, OP.add, OP.max)
                h3s[i, mc] = t
            del h2s[i, 0], h2s[i, 1]

        def emit_l4mm(i):
            p4 = psum.tile([1, B], F32, tag="l4", name=f"p4_{i}", bufs=2)
            for kc in range(2):
                nc.tensor.matmul(
                    p4[:], lhsT=w4c[kc], rhs=h3s[i, kc][:],
                    start=(kc == 0), stop=(kc == 1),
                )
            p4s[i] = p4
            del h3s[i, 0], h3s[i, 1]

        def emit_drain(i):
            # Evacuate the psum score row as fp16 e^(s+b4) (the psum->SBUF
            # hop must be an engine op anyway, so exp rides along free), then
            # DMA it into staging row i (DMA targets any partition; engines
            # only write partition bases {0,32,64,96}).
            s4 = hpool.tile([1, B], MM_DT, tag="s4", name=f"s4_{i}", bufs=3)
            nc.scalar.activation(s4[:], p4s.pop(i)[:], AF.Exp, bias=b4[0:1])
            nc.sync.dma_start(out=staging[i:i + 1, :], in_=s4[:])

        # ---- tail: one batched ln(1+e) over all 64 score rows. Identity/
        # Relu/Exp share the initially-loaded table; only this final Ln
        # forces the one table swap of the kernel (it overlaps the last
        # drain DMA).
        def emit_tail():
            nc.scalar.activation(fin[:], staging[:], AF.Ln, bias=1.0)
            nc.sync.dma_start(out=out_d[:], in_=fin[:])

        emit_l1(0)
        emit_l1(1)
        for t in range(RB + 4):
            if t + 2 < RB:
                emit_l1(t + 2)
            if t >= 4:
                emit_l4mm(t - 4)
                emit_drain(t - 4)
            if 2 <= t < RB + 2:
                emit_l3mm(t - 2)
                emit_l3epi(t - 2)
            if t < RB:
                emit_l2mm(t)
                emit_l2epi(t)
        emit_tail()


def _build_program():
    nc = bacc.Bacc("TRN2", target_bir_lowering=False, debug=False, enable_asserts=False)
    d = {}
    for name, shape, dt in [
        ("blobA", (128, 320), MM_DT),
        ("blobB", (128, 768), MM_DT),
        ("blobC", (128, 1026), MM_DT),
        ("blobD", (128, 7), F32),
    ]:
        d[name] = nc.dram_tensor(name, list(shape), dt, kind="ExternalInput").ap()
    out_d = nc.dram_tensor("out", [RB, B], F32, kind="ExternalOutput").ap()
    with tile.TileContext(nc) as tc:
        _emit(tc, nc, d, out_d)
    nc.compile()
    return nc


_PROGRAM = None


def _get_program():
    global _PROGRAM
    if _PROGRAM is None:
        _PROGRAM = _build_program()
    return _PROGRAM


def _make_in_maps(x, y, W1, b1, W2, b2, W3, b3, W4, b4):
    f = np.float32
    x = np.asarray(x, f)
    y = np.asarray(y, f)
    W1 = np.asarray(W1, f)
    W2 = np.asarray(W2, f)
    W3 = np.asarray(W3, f)
    W4 = np.asarray(W4, f).reshape(H, 1)
    b1 = np.asarray(b1, f).reshape(H)
    b2 = np.asarray(b2, f).reshape(H)
    b3 = np.asarray(b3, f).reshape(H)
    b4v = np.asarray(b4, f).reshape(-1)[0]

    blobB = np.ascontiguousarray(
        np.concatenate([W1[:D], x.T], axis=1), dtype=NP_DT)
    blobC = np.ascontiguousarray(
        np.concatenate([W2[:128], W2[128:], W3[:128], W3[128:],
                        W4[:128], W4[128:]], axis=1), dtype=NP_DT)
    blobD = np.ascontiguousarray(
        np.stack([b1[:128], b1[128:], b2[:128], b2[128:],
                  b3[:128], b3[128:], np.full(128, b4v, f)], axis=1), dtype=f)
    W1b = W1[D:]
    in_maps = []
    for c in range(NCORES):
        blobA = np.ascontiguousarray(
            np.concatenate([y[c * RB:(c + 1) * RB].T, W1b], axis=1), dtype=NP_DT)
        in_maps.append({"blobA": blobA, "blobB": blobB, "blobC": blobC,
                        "blobD": blobD})
    return in_maps


def _run(inputs, trace=False, trace_cores=None):
    nc = _get_program()
    in_maps = _make_in_maps(**inputs)
    res = run_bass_kernel_spmd(
        nc, in_maps, list(range(NCORES)), trace=trace, trace_cores=trace_cores,
    )
    V = np.concatenate([res.results[c]["out"] for c in range(NCORES)], axis=0)
    out = np.ascontiguousarray(V.T, dtype=np.float32)
    return out, res


def kernel(**inputs):
    out, _ = _run(inputs, trace=False)
    return out


# revision 23
# speedup vs baseline: 1.0781x; 1.0056x over previous
"""ConcatCritic MLP over the B^2 pair grid, Trainium2 Bass/Tile kernel.

out[i, j] = softplus(f(x[i], y[j])) where f is a 4-layer MLP on
concat(x, y). Reference pair grid: pairs[a*n+b] = concat(x[b], y[a]),
scores.reshape(n,n).T -> out.

Key factorization: layer 1 is linear in the concat, so
  h1[a,b] = relu(x[b] @ W1top + y[a] @ W1bot + b1)
with W1top = W1[:128], W1bot = W1[128:]. The [B^2, 256] layer-1 matmul
collapses into two tiny matmuls plus a per-partition broadcast add.

Layout: activations kept transposed [features, batch] so every layer's
matmul (weights stationary as lhsT [K, M]) writes the next layer's rhs
directly: out[m=feat, n=j] = sum_k W[k, m] * hT[k, j].

Perf structure (vs the fp32r baseline):
- fp16 matmul operands (same PE stream rate, but FWL weight loads and
  2x DVE read modes; rel err ~1e-3, gate is 2e-2).
- L4 (score row) as two col-tiled M=1 matmuls at tile_position (0,0)
  and (0,32) running concurrently in different column groups -> ~1 PE
  slot instead of 2.
- Per-step drain is a single DMA of the two psum score rows into a
  staging tile; softplus runs batched per 32-row half (one ACT op) --
  no per-step EXP, and Identity/Relu/Softplus all live in one ACT
  table (softplus_and_others) so there are zero mid-kernel table
  swaps.
- PE warm-up: ~40 junk matmuls issued during the input-DMA window keep
  the HAM activity monitor busy so the real stream starts at 2.4 GHz
  (baseline ran its first ~11 us of matmuls at 1.2 GHz).
- Inputs packed into 4 DRAM blobs, triggered on 4 different engine
  queues in parallel.

Sharding: core c owns y rows [c*64, (c+1)*64); computes block
V_c[il, j] = f(x[j], y[c*64+il]) of shape [64, 512]. Host gathers
V = concat(V_c) and returns V.T.

Self-contained: hardcodes shapes; imports concourse from the system repo.
"""

import os
import sys

import numpy as np


def _import_concourse():
    try:
        import concourse  # noqa: F401
        return
    except ImportError:
        pass
    for p in ("/opt/trn_rl_repo", "/root/.axon_site/_ro/trn_rl_repo"):
        if os.path.isdir(p) and p not in sys.path:
            sys.path.insert(0, p)
    import concourse  # noqa: F401


_import_concourse()

import concourse.bacc as bacc  # noqa: E402
import concourse.tile as tile  # noqa: E402
from concourse import mybir  # noqa: E402
from concourse.bass_utils import run_bass_kernel_spmd  # noqa: E402

B = 512          # batch (pair-grid side)
D = 128          # input dim per tensor
H = 256          # hidden dim
NCORES = 8
RB = B // NCORES  # 64 y-rows per core
F32 = mybir.dt.float32
MM_DT = mybir.dt.float16
NP_DT = np.float16
N_WARM = 5       # junk N=512 matmuls to warm the PE HAM before the real stream


def _emit(tc, nc, d, out_d):
    AF = mybir.ActivationFunctionType
    OP = mybir.AluOpType
    from contextlib import ExitStack

    with ExitStack() as ctx:
        const = ctx.enter_context(tc.tile_pool(name="const", bufs=1))
        hpool = ctx.enter_context(tc.tile_pool(name="h", bufs=2))
        psum = ctx.enter_context(tc.tile_pool(name="psum", bufs=1, space="PSUM"))

        # ---- PE warm-up: matmuls on a zeroed junk tile, issued while the
        # input DMAs are in flight. Keeps the HAM activity window busy so
        # the real matmul stream runs at 2.4 GHz from the start.
        junk = const.tile([128, B], MM_DT, tag="junk", name="junk")
        nc.gpsimd.memset(junk[:], 0.0)
        for k in range(N_WARM):
            pw = psum.tile([1, B], F32, tag="l4", name=f"warm{k}", bufs=2)
            nc.tensor.matmul(pw[:], lhsT=junk[:, 0:1], rhs=junk[:],
                             start=True, stop=True)

        # ---- packed input loads, one blob per engine DMA queue.
        A = const.tile([128, 320], MM_DT, tag="blobA", name="A")
        nc.sync.dma_start(out=A[:], in_=d["blobA"][:])
        Bt = const.tile([128, 768], MM_DT, tag="blobB", name="Bt")
        nc.scalar.dma_start(out=Bt[:], in_=d["blobB"][:])
        Dt = const.tile([128, 7], F32, tag="blobD", name="Dt")
        nc.gpsimd.dma_start(out=Dt[:], in_=d["blobD"][:])
        C = const.tile([128, 1026], MM_DT, tag="blobC", name="C")
        nc.gpsimd.dma_start(out=C[:], in_=d["blobC"][:])

        yT = A[:, 0:64]
        w1b = A[:, 64:320]
        w1t = Bt[:, 0:256]
        xT = Bt[:, 256:768]

        def w2(kc, mc):
            return C[:, kc * 256 + mc * 128: kc * 256 + (mc + 1) * 128]

        def w3(kc, mc):
            return C[:, 512 + kc * 256 + mc * 128: 512 + kc * 256 + (mc + 1) * 128]

        w4c = [C[:, 1024:1025], C[:, 1025:1026]]
        b1c = [Dt[:, 0:1], Dt[:, 1:2]]
        b2c = [Dt[:, 2:3], Dt[:, 3:4]]
        b3c = [Dt[:, 4:5], Dt[:, 5:6]]
        b4 = Dt[:, 6:7]

        # ---- preamble: xa[oc] = (x @ W1top)^T + b1 (bias folded here),
        #                yb[oc] = (y_slice @ W1bot)^T
        # Preamble psum tiles borrow the main-loop l2/l3 tags (no extra banks).
        yb = []
        for oc in range(2):
            ms = slice(oc * 128, (oc + 1) * 128)
            pyb = psum.tile([128, B], F32, tag="l3", name=f"pyb{oc}", bufs=3)
            nc.tensor.matmul(pyb[:, 0:RB], lhsT=w1b[:, ms], rhs=yT, start=True, stop=True)
            ybt = const.tile([128, RB], F32, tag=f"yb{oc}", name=f"yb{oc}")
            nc.vector.tensor_copy(ybt[:], pyb[:, 0:RB])
            yb.append(ybt)
        xa = []
        for oc in range(2):
            ms = slice(oc * 128, (oc + 1) * 128)
            pxa = psum.tile([128, B], F32, tag="l2", name=f"pxa{oc}", bufs=3)
            nc.tensor.matmul(pxa[:], lhsT=w1t[:, ms], rhs=xT, start=True, stop=True)
            xat = const.tile([128, B], MM_DT, tag=f"xa{oc}", name=f"xa{oc}")
            nc.scalar.activation(xat[:], pxa[:], AF.Identity, bias=b1c[oc])
            xa.append(xat)

        staging = const.tile([RB, B], MM_DT, tag="stage", name="stage")
        fin = const.tile([RB, B], F32, tag="fin", name="fin")

        # Software-pipelined emission, deep lags so every PE instruction's
        # inputs are produced >= 1 full step earlier (sem latency ~0.4us
        # never exposes): per step t the PE stream is
        #   L4MM(t-4) | L3MM(t-2) | L2MM(t)
        # with L1(t+2) leading 2 steps ahead on DVE.
        # Engine split per step: DVE: 2x L1 + 2x L3epi; ACT: 2x L2epi + exp.
        # Score rows drain by DMA; ln(1+e) batched at the end.
        # PSUM banks: tag "l2" bufs=3 + tag "l3" bufs=3 (each layer's two
        # chunks share a 3-deep rotation -> banks reused 1.5 steps later,
        # long after the epilogue freed them) + l4 bufs=2 = 8 exactly.
        h1s, h2s, h3s, p4s = {}, {}, {}, {}

        def emit_l1(i):
            for oc in range(2):
                t = hpool.tile([128, B], MM_DT, tag=f"h1_{oc}", name=f"h1_{oc}_{i}", bufs=4)
                nc.vector.tensor_scalar(
                    t[:], xa[oc][:], yb[oc][:, i:i + 1], 0.0, OP.add, OP.max
                )
                h1s[i, oc] = t

        def emit_l2mm(i):
            for mc in range(2):
                p = psum.tile([128, B], F32, tag="l2", name=f"p2_{mc}_{i}", bufs=3)
                for kc in range(2):
                    nc.tensor.matmul(
                        p[:], lhsT=w2(kc, mc), rhs=h1s[i, kc][:],
                        start=(kc == 0), stop=(kc == 1),
                    )
                h2s[i, mc] = p  # psum handle; epi converts to SBUF below

        def emit_l2epi(i):
            for mc in range(2):
                p = h2s[i, mc]
                t = hpool.tile([128, B], MM_DT, tag=f"h2_{mc}", name=f"h2_{mc}_{i}", bufs=3)
                nc.scalar.activation(t[:], p[:], AF.Relu, bias=b2c[mc])
                h2s[i, mc] = t

        def emit_l3mm(i):
            for mc in range(2):
                p = psum.tile([128, B], F32, tag="l3", name=f"p3_{mc}_{i}", bufs=3)
                for kc in range(2):
                    nc.tensor.matmul(
                        p[:], lhsT=w3(kc, mc), rhs=h2s[i, kc][:],
                        start=(kc == 0), stop=(kc == 1),
                    )
                h3s[i, mc] = p

        def emit_l3epi(i):
            # Both on DVE: ACT is the busier engine (2x L2epi + exp drain).
            for mc in range(2):
                p = h3s[i, mc]
                t = hpool.tile([128, B], MM_DT, tag=f"h3_{mc}", name=f"h3_{mc}_{i}", bufs=3)
                nc.vector.tensor_scalar(t[:], p[:], b3c[mc], 0.0, OP.add, OP.max)
                h3s[i, mc] = t
            del h2s[i, 0], h2s[i, 1]

        def emit_l4mm(i):
            p4 = psum.tile([1, B], F32, tag="l4", name=f"p4_{i}", bufs=2)
            for kc in range(2):
                nc.tensor.matmul(
                    p4[:], lhsT=w4c[kc], rhs=h3s[i, kc][:],
                    start=(kc == 0), stop=(kc == 1),
                )
            p4s[i] = p4
            del h3s[i, 0], h3s[i, 1]

        def emit_drain(i):
            # Evacuate the psum score row as fp16 e^(s+b4) (the psum->SBUF
            # hop must be an engine op anyway, so exp rides along free), then
            # DMA it into staging row i (DMA targets any partition; engines
            # only write partition bases {0,32,64,96}).
            s4 = hpool.tile([1, B], MM_DT, tag="s4", name=f"s4_{i}", bufs=3)
            nc.scalar.activation(s4[:], p4s.pop(i)[:], AF.Exp, bias=b4[0:1])
            nc.sync.dma_start(out=staging[i:i + 1, :], in_=s4[:])

        # ---- tail: one batched ln(1+e) over all 64 score rows. Identity/
        # Relu/Exp share the initially-loaded table; only this final Ln
        # forces the one table swap of the kernel (it overlaps the last
        # drain DMA).
        def emit_tail():
            nc.scalar.activation(fin[:], staging[:], AF.Ln, bias=1.0)
            nc.sync.dma_start(out=out_d[:], in_=fin[:])

        emit_l1(0)
        emit_l1(1)
        for t in range(RB + 4):
            if t + 2 < RB:
                emit_l1(t + 2)
            if 2 <= t < RB + 2:
                emit_l3mm(t - 2)
                emit_l3epi(t - 2)
            if t >= 4:
                emit_l4mm(t - 4)
                emit_drain(t - 4)
            if t < RB:
                emit_l2mm(t)
                emit_l2epi(t)
        emit_tail()


def _build_program():
    nc = bacc.Bacc("TRN2", target_bir_lowering=False, debug=False, enable_asserts=False)
    d = {}
    for name, shape, dt in [
        ("blobA", (128, 320), MM_DT),
        ("blobB", (128, 768), MM_DT),
        ("blobC", (128, 1026), MM_DT),
        ("blobD", (128, 7), F32),
    ]:
        d[name] = nc.dram_tensor(name, list(shape), dt, kind="ExternalInput").ap()
    out_d = nc.dram_tensor("out", [RB, B], F32, kind="ExternalOutput").ap()
    with tile.TileContext(nc) as tc:
        _emit(tc, nc, d, out_d)
    nc.compile()
    return nc


_PROGRAM = None


def _get_program():
    global _PROGRAM
    if _PROGRAM is None:
        _PROGRAM = _build_program()
    return _PROGRAM


def _make_in_maps(x, y, W1, b1, W2, b2, W3, b3, W4, b4):
    f = np.float32
    x = np.asarray(x, f)
    y = np.asarray(y, f)
    W1 = np.asarray(W1, f)
    W2 = np.asarray(W2, f)
    W3 = np.asarray(W3, f)
    W4 = np.asarray(W4, f).reshape(H, 1)
    b1 = np.asarray(b1, f).reshape(H)
    b2 = np.asarray(b2, f).reshape(H)
    b3 = np.asarray(b3, f).reshape(H)
    b4v = np.asarray(b4, f).reshape(-1)[0]

    blobB = np.ascontiguousarray(
        np.concatenate([W1[:D], x.T], axis=1), dtype=NP_DT)
    blobC = np.ascontiguousarray(
        np.concatenate([W2[:128], W2[128:], W3[:128], W3[128:],
                        W4[:128], W4[128:]], axis=1), dtype=NP_DT)
    blobD = np.ascontiguousarray(
        np.stack([b1[:128], b1[128:], b2[:128], b2[128:],
                  b3[:128], b3[128:], np.full(128, b4v, f)], axis=1), dtype=f)
    W1b = W1[D:]
    in_maps = []
    for c in range(NCORES):
        blobA = np.ascontiguousarray(
            np.concatenate([y[c * RB:(c + 1) * RB].T, W1b], axis=1), dtype=NP_DT)
        in_maps.append({"blobA": blobA, "blobB": blobB, "blobC": blobC,
                        "blobD": blobD})
    return in_maps


def _run(inputs, trace=False, trace_cores=None):
    nc = _get_program()
    in_maps = _make_in_maps(**inputs)
    res = run_bass_kernel_spmd(
        nc, in_maps, list(range(NCORES)), trace=trace, trace_cores=trace_cores,
    )
    V = np.concatenate([res.results[c]["out"] for c in range(NCORES)], axis=0)
    out = np.ascontiguousarray(V.T, dtype=np.float32)
    return out, res


def kernel(**inputs):
    out, _ = _run(inputs, trace=False)
    return out
